# revision 5
# baseline (speedup 1.0000x reference)
"""Multi-head self-attention with RoPE (B=2, S=2048, D=2048, H=16, causal)
on 8 NeuronCores, tensor-parallel over heads (2 heads per core).

Host side: transpose+cast x to bf16, slice per-head weights, build RoPE
cos/sin tables from token_positions, build causal mask tiles. Device does
all matmuls (QKV proj, attention, output proj) in bf16 with f32
accumulation. Row-parallel output projection -> per-core partial outputs
summed on the host.
"""

import numpy as np
import ml_dtypes
from contextlib import ExitStack

import concourse.bass as bass
import concourse.tile as tile
import concourse.mybir as mybir
from concourse import bacc
from concourse.bass_utils import run_bass_kernel_spmd
from concourse.masks import make_identity

bf16 = ml_dtypes.bfloat16
F32 = mybir.dt.float32
BF16 = mybir.dt.bfloat16

# problem dims (hardcoded per spec)
B, S, D, H = 2, 2048, 2048, 16
DK = 128
TOK = B * S                      # 4096 tokens
N_CORES = 8
HPC = H // N_CORES               # heads per core = 2
HD = HPC * DK                    # 256 = per-core head dim
THETA = 10000.0
SCALE = 1.0 / np.sqrt(DK)

TCH = 512                        # token chunk (phase A)
N_TCH = TOK // TCH               # 8
NDCH = D // 128                  # 16 contraction chunks
QCH = 512                        # q chunk (phase B)
NQC = S // QCH                   # 4 per sequence
NKB = S // 128                   # 16 k-blocks per sequence
MASKV = -30000.0


def build_nc(reps: int = 1):
    nc = bacc.Bacc("TRN2", target_bir_lowering=False, debug=False,
                   num_devices=N_CORES)

    xT = nc.dram_tensor("xT", [D, TOK], BF16, kind="ExternalInput")
    wqT = nc.dram_tensor("wqT", [D, HD], BF16, kind="ExternalInput")
    wkT = nc.dram_tensor("wkT", [D, HD], BF16, kind="ExternalInput")
    wvT = nc.dram_tensor("wvT", [D, HD], BF16, kind="ExternalInput")
    woT = nc.dram_tensor("woT", [HD, D], BF16, kind="ExternalInput")
    cosT = nc.dram_tensor("cosT", [DK, TOK], F32, kind="ExternalInput")
    sinT = nc.dram_tensor("sinT", [DK, TOK], F32, kind="ExternalInput")
    maskT = nc.dram_tensor("maskT", [4, 128, QCH], F32, kind="ExternalInput")
    rT = nc.dram_tensor("rT", [DK, DK], BF16, kind="ExternalInput")
    outp = nc.dram_tensor("outp", [TOK, D], F32, kind="ExternalOutput")

    with tile.TileContext(nc) as tc, ExitStack() as ctx:
        consts = ctx.enter_context(tc.tile_pool(name="consts", bufs=1))
        persist = ctx.enter_context(tc.tile_pool(name="persist", bufs=1))
        xin = ctx.enter_context(tc.tile_pool(name="xin", bufs=6))
        sbA = ctx.enter_context(tc.tile_pool(name="sbA", bufs=1))
        sbB = ctx.enter_context(tc.tile_pool(name="sbB", bufs=1))
        sbC = ctx.enter_context(tc.tile_pool(name="sbC", bufs=1))

        # ---- constants ----
        wq_sb = consts.tile([128, NDCH, HD], BF16)
        nc.sync.dma_start(out=wq_sb, in_=wqT.ap().rearrange("(d p) m -> p d m", p=128))
        wk_sb = consts.tile([128, NDCH, HD], BF16)
        nc.sync.dma_start(out=wk_sb, in_=wkT.ap().rearrange("(d p) m -> p d m", p=128))
        wv_sb = consts.tile([128, NDCH, HD], BF16)
        nc.sync.dma_start(out=wv_sb, in_=wvT.ap().rearrange("(d p) m -> p d m", p=128))
        wo_sb = consts.tile([128, HPC, D], BF16)
        nc.sync.dma_start(out=wo_sb, in_=woT.ap().rearrange("(h p) m -> p h m", p=128))
        cos_sb = consts.tile([128, TOK], F32)
        nc.sync.dma_start(out=cos_sb, in_=cosT.ap())
        sin_sb = consts.tile([128, TOK], F32)
        nc.sync.dma_start(out=sin_sb, in_=sinT.ap())
        mask_sb = consts.tile([128, 4, QCH], F32)
        nc.sync.dma_start(out=mask_sb, in_=maskT.ap().rearrange("v p m -> p v m"))
        rt_sb = consts.tile([DK, DK], BF16)
        nc.sync.dma_start(out=rt_sb, in_=rT.ap())
        ident = consts.tile([128, 128], BF16)
        make_identity(nc, ident)
        ones_col = consts.tile([128, 1], BF16)
        nc.vector.memset(ones_col, 1.0)
        ones_row = consts.tile([1, 128], BF16)
        nc.vector.memset(ones_row, 1.0)

        for rep in range(reps):
            # persistent per-head activations (bf16)
            qrot = [persist.tile([128, TOK], BF16, tag=f"qrot{h}", name=f"qrot{h}") for h in range(HPC)]
            krot = [persist.tile([128, TOK], BF16, tag=f"krot{h}", name=f"krot{h}") for h in range(HPC)]
            vnat = [persist.tile([128, TOK], BF16, tag=f"vnat{h}", name=f"vnat{h}") for h in range(HPC)]
            oT = [[persist.tile([128, S], BF16, tag=f"oT{h}_{sq}", name=f"oT{h}_{sq}")
                   for sq in range(B)] for h in range(HPC)]

            # ---- phase A: QKV projections + RoPE + V transpose ----
            with tc.tile_pool(name="apsum", bufs=1, space="PSUM") as apsum:
                for t in range(N_TCH):
                    tsl = bass.ts(t, TCH)
                    ps = {}
                    for m in ("q0", "q1", "k0", "k1", "v0", "v1"):
                        ps[m] = apsum.tile([128, TCH], F32, tag=m, name=f"ps_{m}")
                    wmap = {"q": wq_sb, "k": wk_sb, "v": wv_sb}
                    for d in range(NDCH):
                        xt = xin.tile([128, TCH], BF16, tag="xt")
                        nc.sync.dma_start(out=xt, in_=xT.ap()[d * 128:(d + 1) * 128, tsl])
                        for mk in ("q", "k", "v"):
                            for h in range(HPC):
                                nc.tensor.matmul(
                                    ps[f"{mk}{h}"],
                                    wmap[mk][:, d, h * 128:(h + 1) * 128],
                                    xt,
                                    start=(d == 0), stop=(d == NDCH - 1))
                    # rope for q,k
                    for mk in ("q", "k"):
                        dstm = qrot if mk == "q" else krot
                        for h in range(HPC):
                            raw = sbA.tile([128, TCH], BF16, tag="raw", bufs=3)
                            nc.scalar.copy(out=raw, in_=ps[f"{mk}{h}"])
                            rot_ps = apsum.tile([128, TCH], F32, tag=f"{mk}{h}")
                            nc.tensor.matmul(rot_ps, rt_sb, raw, start=True, stop=True)
                            t1 = sbA.tile([128, TCH], F32, tag="t1", bufs=2)
                            nc.vector.tensor_mul(t1, cos_sb[:, tsl], raw)
                            t2 = sbA.tile([128, TCH], F32, tag="t2", bufs=2)
                            nc.vector.tensor_mul(t2, sin_sb[:, tsl], rot_ps)
                            nc.vector.tensor_add(dstm[h][:, tsl], t1, t2)
                    # v: copy + transpose to natural [tok, dk] blocks
                    for h in range(HPC):
                        vraw = sbA.tile([128, TCH], BF16, tag="vraw", bufs=2)
                        nc.scalar.copy(out=vraw, in_=ps[f"v{h}"])
                        tr_ps = apsum.tile([128, 4, 128], BF16, tag=f"vtr{h}")
                        for s4 in range(4):
                            nc.tensor.transpose(
                                tr_ps[:, s4, :], vraw[:, s4 * 128:(s4 + 1) * 128], ident)
                        nc.scalar.copy(
                            out=vnat[h].rearrange("p (c m) -> p c m", m=128)[:, 4 * t:4 * t + 4, :],
                            in_=tr_ps)

            # ---- phase B + C per sequence ----
            with tc.tile_pool(name="bpsum", bufs=1, space="PSUM") as bp, \
                 tc.tile_pool(name="spsum", bufs=2, space="PSUM") as sp, \
                 tc.tile_pool(name="opsum", bufs=2, space="PSUM") as op, \
                 tc.tile_pool(name="cpsum", bufs=2, space="PSUM") as cp:
                for sq in range(B):
                    for h in range(HPC):
                        for qc in range(NQC):
                            qsl = bass.ds(sq * S + qc * QCH, QCH)
                            nb = 4 * qc + 4
                            o_ps = op.tile([128, QCH], F32, tag="o")
                            lacc = sbB.tile([128, QCH], F32, tag="lacc", bufs=2)
                            for b in range(nb):
                                s_ps = sp.tile([128, QCH], F32, tag="s")
                                nc.tensor.matmul(
                                    s_ps,
                                    krot[h][:, sq * S + b * 128: sq * S + (b + 1) * 128],
                                    qrot[h][:, qsl],
                                    start=True, stop=True)
                                v = b - 4 * qc
                                if v >= 0:
                                    nc.vector.tensor_add(s_ps, s_ps, mask_sb[:, v, :])
                                p_sb = sbB.tile([128, QCH], BF16, tag="p", bufs=3)
                                nc.scalar.activation(
                                    out=p_sb, in_=s_ps,
                                    func=mybir.ActivationFunctionType.Exp, scale=SCALE)
                                nc.tensor.matmul(
                                    o_ps,
                                    vnat[h][:, (sq * NKB + b) * 128:(sq * NKB + b + 1) * 128],
                                    p_sb,
                                    start=(b == 0), stop=(b == nb - 1))
                                if b == 0:
                                    nc.vector.tensor_copy(lacc, p_sb)
                                else:
                                    nc.vector.tensor_add(lacc, lacc, p_sb)
                            # normalization
                            lacc_b = sbB.tile([128, QCH], BF16, tag="laccb", bufs=2)
                            nc.scalar.copy(out=lacc_b, in_=lacc)
                            l_ps = bp.tile([1, QCH], F32, tag="l")
                            nc.tensor.matmul(l_ps, ones_col, lacc_b, start=True, stop=True)
                            rl = sbB.tile([1, QCH], F32, tag="rl", bufs=2)
                            nc.vector.reciprocal(out=rl, in_=l_ps)
                            rl_b = sbB.tile([1, QCH], BF16, tag="rlb", bufs=2)
                            nc.scalar.copy(out=rl_b, in_=rl)
                            bc_ps = bp.tile([128, QCH], F32, tag="bc")
                            nc.tensor.matmul(bc_ps, ones_row, rl_b, start=True, stop=True)
                            bc_sb = sbB.tile([128, QCH], F32, tag="bcs", bufs=2)
                            nc.scalar.copy(out=bc_sb, in_=bc_ps)
                            nc.vector.tensor_mul(
                                oT[h][sq][:, bass.ts(qc, QCH)], o_ps, bc_sb)
                    # ---- phase C: output projection for this sequence ----
                    for tb in range(S // 128):
                        out_sb = sbC.tile([128, D], F32, tag="outsb", bufs=2)
                        for ncn in range(D // 512):
                            o_ps2 = cp.tile([128, 512], F32, tag="c")
                            for h in range(HPC):
                                nc.tensor.matmul(
                                    o_ps2,
                                    oT[h][sq][:, tb * 128:(tb + 1) * 128],
                                    wo_sb[:, h, ncn * 512:(ncn + 1) * 512],
                                    start=(h == 0), stop=(h == HPC - 1))
                            nc.scalar.copy(out=out_sb[:, ncn * 512:(ncn + 1) * 512], in_=o_ps2)
                        nc.sync.dma_start(
                            out=outp.ap()[bass.ds((sq * S // 128 + tb) * 128, 128), :],
                            in_=out_sb)

    nc.compile()
    return nc


def host_prep(x, token_positions):
    """Shared (core-independent) input prep."""
    xT = np.ascontiguousarray(
        x.reshape(TOK, D).T).astype(bf16)

    half = DK // 2
    inv_freq = THETA ** (-np.arange(half, dtype=np.float64) * 2.0 / DK)
    pos = np.asarray(token_positions).astype(np.float64).reshape(TOK)  # [B*S]
    ang = inv_freq[:, None] * pos[None, :]          # [half, TOK]
    cosT = np.repeat(np.cos(ang), 2, axis=0).astype(np.float32)
    sinT = np.repeat(np.sin(ang), 2, axis=0).astype(np.float32)

    # mask variants in [k, q] layout: valid iff (q - 128*v) >= k
    kk = np.arange(128)[:, None]
    qq = np.arange(QCH)[None, :]
    maskT = np.zeros((4, 128, QCH), dtype=np.float32)
    for v in range(4):
        maskT[v] = np.where(qq - 128 * v >= kk, 0.0, MASKV)

    rT = np.zeros((DK, DK), dtype=np.float32)
    for i in range(half):
        # R: qs[2i] = -q[2i+1]; qs[2i+1] = q[2i];  rT = R.T
        rT[2 * i + 1, 2 * i] = -1.0
        rT[2 * i, 2 * i + 1] = 1.0
    rT = rT.astype(bf16)

    return xT, cosT, sinT, maskT, rT


_cached = {}


def _get_nc(reps: int = 1):
    if reps not in _cached:
        _cached[reps] = build_nc(reps)
    return _cached[reps]


def make_in_maps(x, token_positions, Wq, Wk, Wv, Wo):
    xT, cosT, sinT, maskT, rT = host_prep(x, token_positions)
    in_maps = []
    for c in range(N_CORES):
        sl = slice(c * HD, (c + 1) * HD)
        in_maps.append({
            "xT": xT,
            "wqT": np.ascontiguousarray(Wq[sl, :].T).astype(bf16),
            "wkT": np.ascontiguousarray(Wk[sl, :].T).astype(bf16),
            "wvT": np.ascontiguousarray(Wv[sl, :].T).astype(bf16),
            "woT": np.ascontiguousarray(Wo[:, sl].T).astype(bf16),
            "cosT": cosT,
            "sinT": sinT,
            "maskT": maskT,
            "rT": rT,
        })
    return in_maps


def kernel(x, token_positions, Wq, Wk, Wv, Wo, _reps=1, _nc=None):
    x = np.asarray(x, dtype=np.float32)
    Wq = np.asarray(Wq, dtype=np.float32)
    Wk = np.asarray(Wk, dtype=np.float32)
    Wv = np.asarray(Wv, dtype=np.float32)
    Wo = np.asarray(Wo, dtype=np.float32)
    in_maps = make_in_maps(x, token_positions, Wq, Wk, Wv, Wo)
    nc = _nc if _nc is not None else _get_nc(_reps)
    res = run_bass_kernel_spmd(nc, in_maps, core_ids=list(range(N_CORES)))
    out = np.zeros((TOK, D), dtype=np.float32)
    for c in range(N_CORES):
        out += res.results[c]["outp"]
    return out.reshape(B, S, D)


if __name__ == "__main__":
    import reference as ref
    inputs = ref.setup_inputs()
    expected = np.asarray(ref.reference(**inputs))
    got = kernel(**{k: np.asarray(v) for k, v in inputs.items()})
    err = np.abs(got - expected)
    print("absmax err:", err.max())
    print("rel (max/expmax):", err.max() / np.abs(expected).max())
    print("L2 rel:", np.linalg.norm(err) / np.linalg.norm(expected))
